# revision 52
# baseline (speedup 1.0000x reference)
"""Trainium2 Bass kernel for CollaborativeAttention — head-sharded tensor parallel.

Math: with S=512 unique positions and F=T=2048 gathered via fpos/tpos (mod 512),
the whole block collapses to the unique-position problem:
    qf = hs @ Wq ; kf = hs @ Wk ; vf = hs @ Wv + bv ; cbf = hs @ Wcb       [512, *]
    per head h:  w[u, s] = counts[s] * exp(scale*(qf[u]*mix[h]) . kf[s]
                                           + scale*cbf[s, h])
    ctx[u, h*64:(h+1)*64] = (w @ vf[:, h*64:(h+1)*64]) / w.sum(axis=1)
    outfull = ctx @ Wd + bd ; resfull = hs + outfull ; LN  -> normedfull   [512, 1024]
    output  = normedfull[fpos % 512]                                       [2048, 1024]
counts[s] = multiplicity of s in (tpos % 512); softmax over the 2048 keys is
exactly the count-weighted softmax over the 512 unique keys.

Distribution (tensor parallel over H=16 heads, 2 heads per core): every core
computes full q = hs@Wq and k = hs@Wk (the per-head mixing spans all of D, so
scores need the complete q/k), but only its 2 heads' scores/softmax/context,
its 128-column slice of Wv, and its 128-row slice of the output dense Wd.
Each core emits the partial product ctx_slice @ Wd_slice  [512, 1024]; the
all-reduce after the output dense is performed on the host during unshard
(collectives on this stack cost ~1.4 ms, dwarfing the ~40 us kernel), followed
by residual + LayerNorm + the fpos gather.

Precision: the big GEMM blocks (per-head mixed-q projections, k projection,
scores, z/ctx) plus the small v/cb projections run in fp8 e4m3 with DoubleRow
perf mode (2 contraction chunks per pass -> 2x PE throughput); mixing is
folded into per-head Wq on the host (x16 rescale keeps the products in fp8
normal range, compensated in the Exp scale), and the softmax weights carry a
1/8 rescale (folded into the count bias, exactly cancelled by the ctx/z
ratio) so fp8 wt cannot saturate.  The output dense runs in bf16; the
partials ship as fp8 with the Wd slice pre-scaled x64 on the host so they
sit in e4m3 normal range (divided back out after the host-side reduce);
all accumulation is fp32 PSUM; softmax bias/normalization fp32.  Measured
end-to-end rel err 7.5e-3 on hardware vs the fp32 reference (gate is 2e-2;
numpy-simulated quantization of this flow gives 1.5e-3 — hardware fp8
rounding is coarser than the numpy model).

Schedule notes (from TimelineSim traces):
  - Each dma_start costs ~0.6 us of HWDGE descriptor processing, transfers
    serialize on one queue, and a DMA-completion semaphore takes 0.9 us to
    propagate; the transfer order is tuned so the q-projection's inputs
    (wq chunk 0, hT) land exactly as PE needs them, and small constants ride
    behind the critical stream.
  - PE ramps 0.65->1.2->2.4 GHz over ~3 us of continuous execution; dummy
    matmuls during the DMA head buy the ramp before real work arrives.
  - PSUM->SBUF copies of qT/kT alternate between the scalar and vector
    engines so neither becomes the pipeline pace-setter.
  - z/ctx accumulation matmuls trail one score group behind their softmax
    tile so PE never waits on the Exp activation.
"""

import math
import numpy as np

P = 128
S = 512
D = 1024
H = 16
DH = 64
NB = D // P          # 8 contraction chunks
NB2 = NB // 2        # 4 DoubleRow contraction pairs
N_CORES = 8
HPC = H // N_CORES   # 2 heads per core
SCALE = 1.0 / math.sqrt(D / H)  # 0.125
LN_EPS = 1e-5
NEG_BIG = -30000.0
MIX_SCALE = 16.0   # host-folded Wq*diag(mix) rescale to keep fp8 normal-range
W_DESCALE = 8.0    # softmax weights pre-scaled by 1/8 (folded into the count
                   # bias) so fp8 wt can't saturate; cancels in ctx/z ratio
PART_SCALE = 64.0  # Wd slice pre-scaled x64 on host so the fp8 output
                   # partials sit in e4m3 normal range; host divides it back
                   # out after the all-reduce
N_WARM = 6           # dummy matmuls that hide the PE pstate ramp in the DMA head

_CACHE = {}


def _emit(nc, tc, pools, io, it, bv_zero=False):
    """Emit one full compute iteration (everything after the constant loads)."""
    import concourse.mybir as mybir

    fp = mybir.dt.float32
    bf = mybir.dt.bfloat16
    f8 = mybir.dt.float8e4
    DR = mybir.MatmulPerfMode.DoubleRow
    Alu = mybir.AluOpType
    Act = mybir.ActivationFunctionType

    mqp, wp, ps, pss = (pools[k] for k in ("mqp", "wp", "ps", "pss"))
    hT = io["hT"]

    def k2(t, i2, cols=None):
        # [128, 2, *] DoubleRow operand: contraction chunks 2*i2, 2*i2+1
        return t[:, 2 * i2: 2 * i2 + 2, :] if cols is None else \
            t[:, 2 * i2: 2 * i2 + 2, cols]

    # ---- projections (fp8 DoubleRow): mq_h = hs @ (Wq diag(mix_h) * 16)
    # per local head (mixing folded into Wq on host, x16 keeps the fp8
    # values in normal range, compensated in the exp scale), k = hs @ Wk.
    # cb and v (1 DoubleRow matmul each) ride along in the first 16
    # projection groups.
    kT = mqp.tile([P, NB, S], f8, tag="kT", name=f"kT{it}", bufs=1)
    cb_ps = pss.tile([P, 4, HPC], fp, tag="cb", name=f"cb_ps{it}", bufs=1)
    v_ps = pss.tile([P, 4, P], fp, tag="v", name=f"v_ps{it}", bufs=1)
    mq = [mqp.tile([P, NB, S], f8, tag="mq", name=f"mq{it}_{h}", bufs=2)
          for h in range(HPC)]

    # wk in the middle: sc_h0 needs only kT + mq0, so the k projection's
    # trailing PSUM->SBUF copies finish under the wm1 phase and the score
    # phase starts without stalling on them
    projs = ((io["wm0_sb"], mq[0]), (io["wk_sb"], kT),
             (io["wm1_sb"], mq[1]))
    for wi, (wsb, dest) in enumerate(projs):
        for o in range(NB):
            g = NB * wi + o            # group index 0..23
            pt = ps.tile([P, S], fp, tag="ps", name=f"pt{it}")
            for i2 in range(NB2):
                nc.tensor.matmul(pt[:], lhsT=k2(wsb[:, o], i2),
                                 rhs=k2(hT, i2), perf_mode=DR,
                                 start=(i2 == 0), stop=(i2 == NB2 - 1))
            if g < 16:
                # spread work: group g carries the cb and v DoubleRow
                # matmuls for output region st=g%4, contraction pair i2=g//4
                st, i2 = g % 4, g // 4
                hTst = k2(hT, i2, slice(P * st, P * (st + 1)))
                nc.tensor.matmul(cb_ps[:, st, :], lhsT=hTst,
                                 rhs=k2(io["wvcb_sb"], i2, slice(P, P + HPC)),
                                 start=(i2 == 0), stop=(i2 == NB2 - 1),
                                 perf_mode=DR, skip_group_check=True)
                nc.tensor.matmul(v_ps[:, st, :], lhsT=hTst,
                                 rhs=k2(io["wvcb_sb"], i2, slice(0, P)),
                                 start=(i2 == 0), stop=(i2 == NB2 - 1),
                                 perf_mode=DR, skip_group_check=True)
            # psum->sbuf copies alternate scalar/vector so neither engine
            # paces the projection stream
            if g % 2 == 0:
                nc.scalar.copy(dest[:, o, :], pt[:])
            else:
                nc.vector.tensor_copy(dest[:, o, :], pt[:])
        if wi == 1:
            # cb and v complete with group 15; emit their consumers now so
            # they don't queue behind the k projection's DVE copies
            bias_sb = mqp.tile([P, 4, HPC], fp, tag="bias",
                               name=f"bias_sb{it}")
            for st in range(4):
                nc.vector.scalar_tensor_tensor(
                    out=bias_sb[:, st, :], in0=cb_ps[:, st, :], scalar=SCALE,
                    in1=io["lncnt_sb"][:, st:st + 1].to_broadcast([P, HPC]),
                    op0=Alu.mult, op1=Alu.add)
            v_sb = mqp.tile([P, 4, P], f8, tag="v", name=f"v_sb{it}", bufs=1)
            for st in range(4):
                nc.vector.tensor_copy(v_sb[:, st, :], v_ps[:, st, :])

    # ---- scores (fp8 DoubleRow) -> exp -> ctx & Z -> normalize ----
    # The two heads' score groups interleave so the Exp activations (0.61us
    # each on the scalar engine) pace at every-other-group (0.86us) instead
    # of back-to-back; z/ctx accumulate per key-chunk pair in fp8 DoubleRow.
    # ctxn rows 0:64 = head 0 ctx^T, rows 64:128 = head 1 ctx^T; v_sb as lhsT
    # (M=128) lands each head's ctx rows at its own row-half so every DVE op
    # keeps matching partition offsets (lanes have no cross-partition path).
    ctxn = mqp.tile([P, S], bf, tag="ctxn", name=f"ctxn{it}", bufs=1)

    def mk_head(h):
        z_ps = ps.tile([P, S], fp, tag="ps", name=f"zh{it}_{h}")
        ctx_ps = ps.tile([P, S], fp, tag="ps", name=f"ctxh{it}_{h}")
        w_pair = [wp.tile([P, 2, S], f8, tag="w", name=f"wt{it}_{h}_{j}")
                  for j in range(2)]
        return {"z": z_ps, "c": ctx_ps, "w": w_pair, "h": h}

    def sc_group(hs, st):
        h = hs["h"]
        sc = ps.tile([P, S], fp, tag="ps", name=f"sc{it}")
        for i2 in range(NB2):
            nc.tensor.matmul(sc[:],
                             lhsT=k2(kT, i2, slice(P * st, P * (st + 1))),
                             rhs=k2(mq[h], i2), perf_mode=DR,
                             start=(i2 == 0), stop=(i2 == NB2 - 1))
        nc.scalar.activation(hs["w"][st // 2][:, st % 2, :], sc[:], Act.Exp,
                             bias=bias_sb[:, st, h:h + 1],
                             scale=SCALE / MIX_SCALE)

    def zc(hs, j):
        # fp8 DoubleRow over a pair of key chunks; z before ctx: the
        # reciprocal (which only needs z) starts while PE runs ctx
        wt = hs["w"][j]
        nc.tensor.matmul(hs["z"][:], lhsT=io["ones8_sb"][:], rhs=wt[:],
                         start=(j == 0), stop=(j == 1), perf_mode=DR,
                         skip_group_check=True)
        nc.tensor.matmul(hs["c"][:], lhsT=v_sb[:, 2 * j: 2 * j + 2, :],
                         rhs=wt[:], start=(j == 0), stop=(j == 1),
                         perf_mode=DR, skip_group_check=True)

    def norm(hs):
        # (ctx + bv*z) * (1/z) = ctx/z + bv; interleaved per u-half so the
        # out projection's first u-chunk unblocks after one recip+mult
        h = hs["h"]
        rh = DH * h
        rz = mqp.tile([P, S], fp, tag="rz", name=f"rz{it}_{h}", bufs=2)
        HS = S // 2
        if bv_zero:
            for uc in range(2):
                us = slice(HS * uc, HS * (uc + 1))
                nc.vector.reciprocal(rz[rh:rh + DH, us],
                                     hs["z"][rh:rh + DH, us])
                nc.vector.tensor_tensor(ctxn[rh:rh + DH, us],
                                        hs["c"][rh:rh + DH, us],
                                        rz[rh:rh + DH, us], Alu.mult)
        else:
            cbv = mqp.tile([P, S], fp, tag="cbv", name=f"cbv{it}_{h}",
                           bufs=2)
            for uc in range(2):
                us = slice(HS * uc, HS * (uc + 1))
                nc.vector.reciprocal(rz[rh:rh + DH, us],
                                     hs["z"][rh:rh + DH, us])
                nc.vector.scalar_tensor_tensor(
                    out=cbv[rh:rh + DH, us], in0=hs["z"][rh:rh + DH, us],
                    scalar=io["bv_sb"][rh:rh + DH, 0:1],
                    in1=hs["c"][rh:rh + DH, us], op0=Alu.mult, op1=Alu.add)
                nc.vector.tensor_tensor(ctxn[rh:rh + DH, us],
                                        cbv[rh:rh + DH, us],
                                        rz[rh:rh + DH, us], Alu.mult)

    # software-pipelined heads: head 0's closing z/ctx pair (which waits on
    # its last Exp) is deferred until after head 1's first score group, so
    # PE fills the activation latency with useful work; zc for pair 0 trails
    # behind each head's st=2 score group for the same reason
    hs0 = mk_head(0)
    for st in range(4):
        sc_group(hs0, st)
        if st == 2:
            zc(hs0, 0)
    hs1 = mk_head(1)
    sc_group(hs1, 0)
    zc(hs0, 1)
    norm(hs0)
    for st in range(1, 4):
        sc_group(hs1, st)
        if st == 2:
            zc(hs1, 0)
    zc(hs1, 1)
    norm(hs1)

    # ---- partial output projection: ctx_slice @ Wd_slice -> [512, 1024] ----
    # Single contraction chunk (this core's 128 d-columns); bf16 partials,
    # one stage buffer per u-chunk so copies and DMAs fully pipeline.
    for ut in range(4):
        stage = mqp.tile([P, D], f8, tag="stage", name=f"stage{it}", bufs=4)
        for eh in range(2):
            po = ps.tile([P, S], fp, tag="ps", name=f"po{it}")
            nc.tensor.matmul(po[:], lhsT=ctxn[:, P * ut: P * (ut + 1)],
                             rhs=io["wd_sb"][:, S * eh: S * (eh + 1)],
                             start=True, stop=True)
            if eh == 0:
                nc.scalar.copy(stage[:, 0:S], po[:])
            else:
                nc.vector.tensor_copy(stage[:, S:D], po[:])
        # all out DMAs on the SP queue: a scalar-queue dma_start would eat
        # ~0.7us of Activation SEQ right when it runs the tail copies
        nc.sync.dma_start(io["outp"][P * ut: P * (ut + 1), :], stage[:])


def _build(iters=1, bv_zero=False):
    import concourse.mybir as mybir
    import concourse.tile as tile
    from concourse import bacc

    fp = mybir.dt.float32
    bf = mybir.dt.bfloat16
    f8 = mybir.dt.float8e4

    nc = bacc.Bacc("TRN2", target_bir_lowering=False, debug=False,
                   num_devices=N_CORES)

    # Host-prepacked layouts (see _prepare_in_maps): every DMA line below is
    # contiguous per partition.
    hTp = nc.dram_tensor("hTp", [P, NB, S], f8, kind="ExternalInput").ap()
    wm0p = nc.dram_tensor("wm0p", [NB, P, NB, P], f8,
                          kind="ExternalInput").ap()
    wm1p = nc.dram_tensor("wm1p", [NB, P, NB, P], f8,
                          kind="ExternalInput").ap()
    wkp = nc.dram_tensor("wkp", [NB, P, NB, P], f8, kind="ExternalInput").ap()
    # Wv slice (128 cols) and Wcb slice (2 cols) packed: [p, ic, 130]
    wvcbp = nc.dram_tensor("wvcbp", [P, NB, P + HPC], f8,
                           kind="ExternalInput").ap()
    wdp = nc.dram_tensor("wdp", [P, D], bf, kind="ExternalInput").ap()
    # ln(counts) [p, 4] and bv slice [p, 1] packed together: [p, 5]
    cstf = nc.dram_tensor("cstf", [P, 5], fp, kind="ExternalInput").ap()
    outp = nc.dram_tensor("outp", [S, D], f8, kind="ExternalOutput").ap()

    def flat(ap):
        return ap.rearrange("p a b -> p (a b)")

    with tile.TileContext(nc) as tc:
        with (
            tc.tile_pool(name="singles", bufs=1) as singles,
            tc.tile_pool(name="mqp", bufs=2) as mqp,
            tc.tile_pool(name="wp", bufs=8) as wp,
            tc.tile_pool(name="ps", bufs=6, space="PSUM") as ps,
            tc.tile_pool(name="pss", bufs=1, space="PSUM") as pss,
        ):
            pools = {"singles": singles, "mqp": mqp, "wp": wp, "ps": ps,
                     "pss": pss}
            # ---- input loads; issue order == transfer order, tuned so the
            # q projection streams without stalls: first wq chunk and hT in
            # interleaved halves, wq chunks 1-7, then wv+wcb/wk/wd/constants
            wm0_sb = singles.tile([P, NB, NB, P], f8)
            wm1_sb = singles.tile([P, NB, NB, P], f8)
            wk_sb = singles.tile([P, NB, NB, P], f8)
            hT = singles.tile([P, NB, S], f8)

            def wchunk(dst_sb, src, o0, o1):
                nc.sync.dma_start(
                    dst_sb[:, o0:o1, :, :].rearrange("p o a b -> p o (a b)"),
                    src[o0:o1, :, :, :].rearrange("o p a b -> p o (a b)"))

            nc.sync.dma_start(
                wm0_sb[:, 0, 0:4, :].rearrange("p a b -> p (a b)"),
                wm0p[0:1, :, 0:4, :].rearrange("o p a b -> p (o a b)"))
            nc.sync.dma_start(flat(hT[:, 0:4, :]), flat(hTp[:, 0:4, :]))
            nc.sync.dma_start(
                wm0_sb[:, 0, 4:8, :].rearrange("p a b -> p (a b)"),
                wm0p[0:1, :, 4:8, :].rearrange("o p a b -> p (o a b)"))
            nc.sync.dma_start(flat(hT[:, 4:8, :]), flat(hTp[:, 4:8, :]))
            # 2-3 chunk granules: each dma_start costs ~0.62 us of HWDGE
            # descriptor processing, so per-chunk DMAs would cap delivery at
            # ~0.65 us/chunk while PE consumes one every ~0.45 us
            for o0, o1 in ((1, 3), (3, 5), (5, 8)):
                wchunk(wm0_sb, wm0p, o0, o1)
            wvcb_sb = singles.tile([P, NB, P + HPC], f8)
            nc.sync.dma_start(flat(wvcb_sb), flat(wvcbp))
            cstf_sb = singles.tile([P, 5], fp)
            nc.sync.dma_start(cstf_sb[:], cstf)
            for o0, o1 in ((0, 2), (2, 4), (4, 6), (6, 8)):
                wchunk(wk_sb, wkp, o0, o1)
            for o0, o1 in ((0, 2), (2, 4), (4, 6), (6, 8)):
                wchunk(wm1_sb, wm1p, o0, o1)
            wd_sb = singles.tile([P, D], bf)
            nc.sync.dma_start(wd_sb[:], wdp)

            ones_sb = singles.tile([P, P], bf)
            nc.vector.memset(ones_sb[:], 1.0)
            ones8_sb = singles.tile([P, 2, P], f8)
            nc.vector.memset(ones8_sb[:], 1.0)
            zeros_sb = singles.tile([P, S], bf)
            nc.gpsimd.memset(zeros_sb[:], 0.0)

            # PE pstate warmup: dummy matmuls while the first DMAs land
            warm = ps.tile([P, S], fp, tag="ps", name="warm")
            for _ in range(N_WARM):
                nc.tensor.matmul(warm[:], lhsT=ones_sb[:], rhs=zeros_sb[:],
                                 start=True, stop=True)

            io = {"hT": hT, "wm0_sb": wm0_sb, "wm1_sb": wm1_sb,
                  "wk_sb": wk_sb, "wvcb_sb": wvcb_sb, "wd_sb": wd_sb,
                  "bv_sb": cstf_sb[:, 4:5], "lncnt_sb": cstf_sb[:, 0:4],
                  "ones_sb": ones_sb, "ones8_sb": ones8_sb, "outp": outp}

            with nc.allow_low_precision(reason="fp8/bf16 matmul path, "
                                        "fp32 psum accumulation"):
                for it in range(iters):
                    _emit(nc, tc, pools, io, it, bv_zero=bv_zero)

    nc.compile()
    return nc


def _get_nc(iters=1, bv_zero=False):
    key = ("nc", iters, bv_zero)
    if key not in _CACHE:
        _CACHE[key] = _build(iters, bv_zero=bv_zero)
    return _CACHE[key]


def _prepare_in_maps(hidden_states, fpos, tpos, Wq, Wk, Wcb, Wv, bv, mixing,
                     Wd, bd, ln_gamma, ln_beta):
    import ml_dtypes
    bf = ml_dtypes.bfloat16
    f8 = ml_dtypes.float8_e4m3

    hs = np.ascontiguousarray(np.asarray(hidden_states, dtype=np.float32))
    tidx = np.asarray(tpos).astype(np.int64) % S
    counts = np.bincount(tidx, minlength=S).astype(np.float64)
    lncnt = np.where(counts > 0,
                     np.log(np.maximum(counts, 1e-30) / W_DESCALE),
                     NEG_BIG).astype(np.float32)

    def pack_w(w):  # [D, D] -> [o, p, ic, m] with w[ic*128+p, o*128+m]
        return np.ascontiguousarray(
            np.asarray(w, np.float32).astype(f8)
            .reshape(NB, P, NB, P).transpose(2, 1, 0, 3))

    def pack_dvec(w, dt):  # [D, n] -> [p, ic, n]
        n = w.shape[1]
        return np.ascontiguousarray(
            np.asarray(w, np.float32).astype(dt)
            .reshape(NB, P, n).transpose(1, 0, 2))

    hTp = np.ascontiguousarray(
        hs.T.astype(f8).reshape(NB, P, S).transpose(1, 0, 2))
    wkp = pack_w(Wk)
    Wq32 = np.asarray(Wq, np.float64)
    lncp = lncnt.reshape(4, P).T  # [p, 4]
    Wv32 = np.asarray(Wv, np.float32)
    Wd32 = np.asarray(Wd, np.float32)
    Wcb32 = np.asarray(Wcb, np.float32)
    mix32 = np.asarray(mixing, np.float32)
    bv32 = np.asarray(bv, np.float32)

    in_maps = []
    for c in range(N_CORES):
        cs = P * c
        wvcb = np.concatenate(
            [Wv32[:, cs:cs + P], Wcb32[:, HPC * c: HPC * (c + 1)]], axis=1)
        cstf = np.concatenate([lncp, bv32[cs:cs + P].reshape(P, 1)], axis=1)
        wm = [pack_w(Wq32 * (mix32[HPC * c + h][None, :] * MIX_SCALE))
              for h in range(HPC)]
        in_maps.append({
            "hTp": hTp,
            "wm0p": wm[0],
            "wm1p": wm[1],
            "wkp": wkp,
            "wvcbp": pack_dvec(wvcb, f8),
            "wdp": np.ascontiguousarray(
                (Wd32[cs:cs + P, :] * PART_SCALE).astype(bf)),
            "cstf": np.ascontiguousarray(cstf.astype(np.float32)),
        })
    return in_maps


def _finish_host(partials, inputs):
    """All-reduce of the per-core output-dense partials + residual + LN +
    fpos gather (the unshard step)."""
    hs = np.asarray(inputs["hidden_states"], np.float32)
    out = np.zeros((S, D), np.float64)
    for p in partials:
        out += np.asarray(p, np.float64)
    out /= PART_SCALE
    res = hs.astype(np.float64) + out + np.asarray(inputs["bd"], np.float64)
    mu = res.mean(axis=-1, keepdims=True)
    var = res.var(axis=-1, keepdims=True)
    normed = (res - mu) / np.sqrt(var + LN_EPS)
    normed = (normed * np.asarray(inputs["ln_gamma"], np.float64)
              + np.asarray(inputs["ln_beta"], np.float64)).astype(np.float32)
    fidx = np.asarray(inputs["fpos"]).astype(np.int64) % S
    return np.ascontiguousarray(normed[fidx])


def _run(inputs, trace=False, iters=1):
    from concourse import bass_utils
    bv_zero = bool(np.all(np.asarray(inputs["bv"]) == 0.0))
    nc = _get_nc(iters, bv_zero=bv_zero)
    in_maps = _prepare_in_maps(**inputs)
    res = bass_utils.run_bass_kernel_spmd(
        nc, in_maps, core_ids=list(range(N_CORES)), trace=trace)
    partials = [res.results[c]["outp"] for c in range(N_CORES)]
    return _finish_host(partials, inputs), res


def kernel(**inputs) -> np.ndarray:
    out, _ = _run(inputs, trace=False)
    return out
